# revision 1
# baseline (speedup 1.0000x reference)
"""GCN message-passing kernel for 8 Trainium2 NeuronCores (Bass/Tile).

Strategy (SPMD, one program for all 8 cores):
  - Nodes sharded contiguously: core c owns nodes [5000c, 5000(c+1)), padded
    to 5120 (40 blocks of 128).  Within the shard, nodes are permuted by a
    greedy load-balancer so every (core, block) has bounded in-degree.
  - Edges assigned to the core owning their dst, grouped into 128-edge tiles
    per dst-block (8 tiles per block).
  - Gather: per-tile indirect DMA pulls 128 message rows from a replicated
    node-major table in HBM (bf16).  Scatter: one-hot matmul on the
    TensorEngine accumulating into PSUM (edge-tile stationary = transposed
    output for conv1/2; one-hot stationary = node-major output for conv3).
  - All per-edge scalars (ew * in_inv[dst] * out_inv[src]) fold into the
    one-hot coefficient.  GraphConv weight is applied before propagation
    when it shrinks the message (conv2: 512->256, conv3: 256->128).
  - Dense transforms run feature-major with the weights stationary.
    LayerNorm stats use ones-matmul partition reduction + K=1 broadcast.
  - Tables AllGather'd between convs; readout AllReduce'd; final L2
    normalize computed identically on every core.
"""
import os
import numpy as np
import ml_dtypes

import concourse.bacc as bacc
import concourse.bass as bass
import concourse.tile as tile
import concourse.mybir as mybir
import concourse.bass_utils as bass_utils
from concourse.bass import IndirectOffsetOnAxis

# ---------------- problem constants (hardcoded per spec) ----------------
N_NODES = 40000
N_EDGES = 320000
N_GRAPHS = 64
IN_DIM = 128
HID4 = 256
OUT_DIM = 128
LN_EPS = 1e-5

NCORES = 8
SH = N_NODES // NCORES          # 5000 nodes per core
NBLK = 40                       # 128-node blocks per core
P = 128
SHP = NBLK * P                  # 5120 padded nodes per core
VP = NCORES * SHP               # 40960 padded global rows
TB = 8                          # tiles per block
SBB = 4                         # blocks per superblock (gather chunk)
NSB = NBLK // SBB               # 10 superblocks
CHUNK = SBB * P                 # 512 nodes per dense chunk
TILES_SB = SBB * TB             # 32 tiles per superblock
NT = NBLK * TB                  # 320 tiles per core

F32 = mybir.dt.float32
BF16 = mybir.dt.bfloat16
I32 = mybir.dt.int32
BF = ml_dtypes.bfloat16

AF = mybir.ActivationFunctionType
OP = mybir.AluOpType


# ======================= host-side preprocessing =======================

def _preprocess(x, w, src, dst, graph_ids):
    x = np.asarray(x, np.float32)
    w = np.asarray(w, np.float32)
    src = np.asarray(src, np.int64)
    dst = np.asarray(dst, np.int64)
    graph_ids = np.asarray(graph_ids, np.int64)

    deg_out = np.bincount(src, minlength=N_NODES).astype(np.float64)
    deg_in = np.bincount(dst, minlength=N_NODES).astype(np.float64)
    out_inv = (1.0 / np.sqrt(np.maximum(deg_out, 1.0))).astype(np.float32)
    in_inv = (1.0 / np.sqrt(np.maximum(deg_in, 1.0))).astype(np.float32)

    # ---- per-core node -> (block, local) assignment, balancing in-degree ----
    slot_of = np.full(N_NODES, -1, np.int64)     # slot in [0, SHP) within shard
    for c in range(NCORES):
        lo, hi = c * SH, (c + 1) * SH
        em = (dst >= lo) & (dst < hi)
        tot = np.bincount(dst[em] - lo, minlength=SH)
        order = np.argsort(-tot, kind="stable")
        loads = np.zeros(NBLK, np.int64)
        counts = np.zeros(NBLK, np.int64)
        blk = np.empty(SH, np.int64)
        loc = np.empty(SH, np.int64)
        for v in order:
            masked = np.where(counts < P, loads, 1 << 60)
            b = int(np.argmin(masked))
            blk[v] = b
            loc[v] = counts[b]
            counts[b] += 1
            loads[b] += tot[v]
        assert loads.max() <= TB * P, f"core {c}: max block load {loads.max()}"
        slot_of[lo:hi] = blk * P + loc

    core_of = np.arange(N_NODES) // SH
    allslot = slot_of[np.arange(N_NODES)]
    QSH = SHP // 4
    quart = allslot // QSH
    # quarter-major layout: AllGather of each shard-quarter writes one slice
    rowp = quart * (VP // 4) + core_of * QSH + (allslot % QSH)

    # ---- replicated inputs ----
    x_bf = np.zeros((VP, IN_DIM), BF)
    x_bf[rowp] = x.astype(BF)
    iota128 = np.tile(np.arange(P, dtype=np.float32), (P, 1))
    ones_col = np.ones((P, 1), np.float32)
    ones_row = np.ones((1, P), np.float32)

    per_core = []
    for c in range(NCORES):
        lo, hi = c * SH, (c + 1) * SH
        em_idx = np.nonzero((dst >= lo) & (dst < hi))[0]
        e_dst = dst[em_idx]
        e_slot = slot_of[e_dst]
        e_blk = e_slot // P
        e_dl = (e_slot % P).astype(np.float32)
        e_row = rowp[src[em_idx]]
        e_q = in_inv[e_dst] * out_inv[src[em_idx]]

        # order edges by block, then place into slot grid [p, t]
        order = np.argsort(e_blk, kind="stable")
        gidx = np.zeros((P, NT), np.int32)
        dstl = np.zeros((P, NT), np.float32)
        qv = np.zeros((P, NT), np.float32)
        wdat = np.zeros((P, NT, 4), np.float32)
        bstart = np.searchsorted(e_blk[order], np.arange(NBLK + 1))
        for b in range(NBLK):
            sel = order[bstart[b]:bstart[b + 1]]
            k = np.arange(len(sel))
            t = b * TB + k // P
            p = k % P
            gidx[p, t] = e_row[sel]
            dstl[p, t] = e_dl[sel]
            qv[p, t] = e_q[sel]
            wdat[p, t] = w[em_idx[sel]]

        nodes = np.arange(lo, hi)
        slots = slot_of[nodes]
        xT = np.zeros((IN_DIM, SHP), np.float32)
        xT[:, slots] = x[nodes].T
        gid = np.zeros((P, NBLK), np.float32)
        gid[slots % P, slots // P] = graph_ids[nodes]

        xe = x_bf[gidx.reshape(-1)].reshape(P, NT * IN_DIM)
        per_core.append(dict(
            gidx=gidx, dstl=dstl, q=qv,
            wdat=wdat.reshape(P, NT * 4),
            xT=xT, gid=gid, xe=xe,
        ))
    shared = dict(x_bf=x_bf, iota=iota128, ones_col=ones_col,
                  ones_row=ones_row)
    return shared, per_core


# ======================= device kernel =======================

def _conv_scatter(tc, pools, cdat, table, dnum, transposed,
                  post_block, post_sb, dtt=BF16, stream_src=None):
    """Shared conv loop: gathers (indirect, or a contiguous host-expanded
    stream) + one-hot scatter matmuls."""
    nc = tc.nc
    gp, ohp, aggp = pools["gather"], pools["onehot"], pools["agg"]
    gidx, dstl_sb, c_sb, iota_sb = (cdat["gidx"], cdat["dstl"],
                                    cdat["c"], cdat["iota"])
    ndb = dnum // P  # feature blocks per message
    for s in range(NSB):
        g = gp.tile([P, TILES_SB, dnum], dtt, tag="gbuf")
        if stream_src is not None:
            w0 = s * TILES_SB * dnum
            nc.sync.dma_start(g[:].rearrange("p t d -> p (t d)"),
                              stream_src[:, w0:w0 + TILES_SB * dnum])
        else:
            for sl in range(TILES_SB):
                t_col = s * TILES_SB + sl
                nc.gpsimd.indirect_dma_start(
                    out=g[:, sl, :], out_offset=None, in_=table,
                    in_offset=IndirectOffsetOnAxis(
                        ap=gidx[:, t_col:t_col + 1], axis=0))
        if transposed:
            agg = [aggp.tile([P, SBB * P], F32, space="PSUM", tag="agg",
                             name=f"agg{db}") for db in range(ndb)]
        for bi in range(SBB):
            b = s * SBB + bi
            if not transposed:
                agg = aggp.tile([P, P], F32, space="PSUM", tag="agg",
                                name="aggnm")
            for j in range(TB):
                sl = bi * TB + j
                t_col = s * TILES_SB + sl
                oh = ohp.tile([P, P], dtt, tag="oh")
                nc.vector.tensor_scalar(
                    out=oh[:], in0=iota_sb[:, :P],
                    scalar1=dstl_sb[:, t_col:t_col + 1],
                    scalar2=c_sb[:, t_col:t_col + 1],
                    op0=OP.is_equal, op1=OP.mult)
                msg = g[:, sl, :]
                if transposed:
                    for db in range(ndb):
                        nc.tensor.matmul(
                            agg[db][:, bi * P:(bi + 1) * P],
                            lhsT=msg[:, db * P:(db + 1) * P], rhs=oh[:],
                            start=(j == 0), stop=(j == TB - 1))
                else:
                    nc.tensor.matmul(
                        agg[:], lhsT=oh[:], rhs=msg,
                        start=(j == 0), stop=(j == TB - 1))
            if post_block is not None:
                post_block(s, bi, agg)
        if post_sb is not None:
            post_sb(s, agg)


def build_kernel(tc, ins, outs):
    nc = tc.nc
    out_ap = outs["out"][:]

    # internal DRAM tensors
    y2nm = nc.dram_tensor("y2nm", [SHP, HID4], BF16, kind="Internal").ap()
    y3nm = nc.dram_tensor("y3nm", [SHP, OUT_DIM], BF16, kind="Internal").ap()
    table2 = nc.dram_tensor("table2", [VP, HID4], BF16, kind="Internal",
                            addr_space="Shared").ap()
    table3 = nc.dram_tensor("table3", [VP, OUT_DIM], BF16, kind="Internal",
                            addr_space="Shared").ap()
    ro_in = nc.dram_tensor("ro_in", [N_GRAPHS, OUT_DIM], F32,
                           kind="Internal").ap()
    ro_out = nc.dram_tensor("ro_out", [N_GRAPHS, OUT_DIM], F32,
                            kind="Internal", addr_space="Shared").ap()
    rg = [list(range(NCORES))]

    with tc.tile_pool(name="const", bufs=1) as cp, \
         tc.tile_pool(name="gather", bufs=3) as gp, \
         tc.tile_pool(name="onehot", bufs=8) as ohp, \
         tc.tile_pool(name="work", bufs=2) as wp, \
         tc.tile_pool(name="chunk", bufs=2) as chp, \
         tc.tile_pool(name="agg", bufs=2, space="PSUM") as aggp, \
         tc.tile_pool(name="dense", bufs=4, space="PSUM") as dp, \
         tc.tile_pool(name="stats", bufs=1, space="PSUM") as sp, \
         tc.tile_pool(name="ro", bufs=1, space="PSUM") as rop:

        pools = dict(gather=gp, onehot=ohp, agg=aggp)

        # ---- load constants ----
        def cload(name, shape, dt):
            t = cp.tile(shape, dt, name=name, tag=name)
            nc.sync.dma_start(t[:], ins[name][:])
            return t

        iota_sb = cload("iota", [P, P], F32)
        onesc = cload("ones_col", [P, 1], F32)
        onesr = cload("ones_row", [1, P], F32)
        W1 = cload("W1", [IN_DIM, HID4], F32)
        fc1W = cload("fc1_W", [IN_DIM, HID4], F32)
        W2r = cload("W2r", [P, 4 * HID4], F32)
        W3r = cload("W3r", [P, 2 * OUT_DIM], F32)
        gammaT = cload("gammaT", [P, 2], F32)
        betaT = cload("betaT", [P, 2], F32)
        gidx_sb = cload("gidx", [P, NT], I32)
        dstl_sb = cload("dstl", [P, NT], F32)
        q_sb = cload("q", [P, NT], F32)
        wdat_sb = cload("wdat", [P, NT * 4], F32)
        gid_sb = cload("gid", [P, NBLK], F32)
        xT_dram = ins["xT"]

        # ---- c = max(w, axis=1) * q ----
        eps_t = cp.tile([1, 1], F32)
        nc.vector.memset(eps_t[:], LN_EPS)

        ew_sb = cp.tile([P, NT], F32)
        nc.vector.tensor_reduce(
            out=ew_sb[:], in_=wdat_sb[:].rearrange("p (t f) -> p t f", f=4),
            axis=mybir.AxisListType.X, op=OP.max)
        c_sb = cp.tile([P, NT], F32)
        nc.vector.tensor_tensor(out=c_sb[:], in0=ew_sb[:], in1=q_sb[:],
                                op=OP.mult)

        cdat = dict(gidx=gidx_sb[:], dstl=dstl_sb[:], c=c_sb[:],
                    iota=iota_sb[:])

        wbar = cp.tile([P, 1], F32, name="wbar", tag="wbar")
        nc.vector.tensor_reduce(out=wbar[:], in_=fc1W[:],
                                axis=mybir.AxisListType.X, op=OP.add)

        # =========== phase 1: conv1 + fc1 + y2' (fused per superblock) =====
        def p1_post_sb(s, agg_ps):
            n0 = s * CHUNK
            # conv1 agg -> SBUF
            a1 = wp.tile([P, CHUNK], F32, tag="a1")
            nc.vector.tensor_copy(a1[:], agg_ps[0][:])
            # x1T = relu(W1^T @ a1)  (2 feature blocks)
            x1c = [chp.tile([P, CHUNK], F32, tag="x1c", name=f"x1c{ob}") for ob in range(2)]
            for ob in range(2):
                ps = dp.tile([P, CHUNK], F32, space="PSUM", tag="dps")
                nc.tensor.matmul(ps[:], lhsT=W1[:, ob * P:(ob + 1) * P],
                                 rhs=a1[:], start=True, stop=True)
                nc.scalar.activation(x1c[ob][:], ps[:], AF.Relu)
            # fc1 chunk
            xTc = wp.tile([P, CHUNK], F32, tag="xTc")
            nc.sync.dma_start(xTc[:], xT_dram[:, n0:n0 + CHUNK])
            fpre = [dp.tile([P, CHUNK], F32, space="PSUM", tag="dps", name=f"fpre{ob}")
                    for ob in range(2)]
            fps = [wp.tile([P, CHUNK], F32, tag="fp", name=f"fp{ob}") for ob in range(2)]
            sqs = [wp.tile([P, CHUNK], F32, tag="sq", name=f"sq{ob}") for ob in range(2)]
            for ob in range(2):
                nc.tensor.matmul(fpre[ob][:], lhsT=fc1W[:, ob * P:(ob + 1) * P],
                                 rhs=xTc[:], start=True, stop=True)
                nc.scalar.copy(fps[ob][:], fpre[ob][:])
                nc.vector.tensor_tensor(out=sqs[ob][:], in0=fps[ob][:],
                                        in1=fps[ob][:], op=OP.mult)
            srow = wp.tile([1, 2 * CHUNK], F32, tag="srow")
            stats = sp.tile([1, CHUNK], F32, space="PSUM", tag="stats",
                            name="stats_s")
            nc.tensor.matmul(stats[:], lhsT=wbar[:], rhs=xTc[:],
                             start=True, stop=True)
            nc.vector.tensor_copy(srow[:, :CHUNK], stats[:])
            stats2 = rop.tile([1, CHUNK], F32, space="PSUM", tag="ro_ps",
                              name="stats_ss")
            for ob in range(2):
                nc.tensor.matmul(stats2[:], lhsT=onesc[:], rhs=sqs[ob][:],
                                 start=(ob == 0), stop=(ob == 1))
            nc.vector.tensor_copy(srow[:, CHUNK:], stats2[:])
            # lane-0 stats math
            mu1 = wp.tile([1, CHUNK], F32, tag="mu1")
            var1 = wp.tile([1, CHUNK], F32, tag="var1")
            nc.vector.tensor_scalar(out=mu1[:], in0=srow[:, :CHUNK],
                                    scalar1=1.0 / HID4, scalar2=None,
                                    op0=OP.mult)
            nc.vector.tensor_scalar(out=var1[:], in0=srow[:, CHUNK:],
                                    scalar1=1.0 / HID4, scalar2=None,
                                    op0=OP.mult)
            musq = wp.tile([1, CHUNK], F32, tag="musq")
            nc.vector.tensor_tensor(out=musq[:], in0=mu1[:], in1=mu1[:],
                                    op=OP.mult)
            nc.vector.tensor_tensor(out=var1[:], in0=var1[:], in1=musq[:],
                                    op=OP.subtract)
            lnv = wp.tile([1, CHUNK], F32, tag="lnv")
            nc.scalar.activation(lnv[:], var1[:], AF.Ln, bias=eps_t[:1, :1])
            rstd1 = wp.tile([1, CHUNK], F32, tag="rstd1")
            nc.scalar.activation(rstd1[:], lnv[:], AF.Exp, scale=-0.5)
            # broadcast mu and rstd to 128 partitions
            mub = wp.tile([P, CHUNK], F32, tag="mub")
            rstdb = wp.tile([P, CHUNK], F32, tag="rstdb")
            bcm = dp.tile([P, CHUNK], F32, space="PSUM", tag="dps",
                          name="bcm")
            nc.tensor.matmul(bcm[:], lhsT=onesr[:], rhs=mu1[:],
                             start=True, stop=True)
            nc.scalar.copy(mub[:], bcm[:])
            bcr = dp.tile([P, CHUNK], F32, space="PSUM", tag="dps",
                          name="bcr")
            nc.tensor.matmul(bcr[:], lhsT=onesr[:], rhs=rstd1[:],
                             start=True, stop=True)
            nc.vector.tensor_copy(rstdb[:], bcr[:])
            f1c = [chp.tile([P, CHUNK], F32, tag="f1c", name=f"f1c{ob}") for ob in range(2)]
            for ob in range(2):
                d = wp.tile([P, CHUNK], F32, tag="lnd")
                nc.vector.tensor_tensor(out=d[:], in0=fps[ob][:], in1=mub[:],
                                        op=OP.subtract)
                nc.vector.tensor_tensor(out=d[:], in0=d[:], in1=rstdb[:],
                                        op=OP.mult)
                nc.scalar.activation(f1c[ob][:], d[:], AF.Relu,
                                     bias=betaT[:, ob:ob + 1],
                                     scale=gammaT[:, ob:ob + 1])
            # y2' node-major: per node-block, x1f1^T blocks stationary
            lhs_k = [x1c[0], x1c[1], f1c[0], f1c[1]]
            for bi in range(SBB):
                ps = dp.tile([P, HID4], F32, space="PSUM", tag="dps",
                             name="y2ps")
                for kb in range(4):
                    nc.tensor.matmul(
                        ps[:], lhsT=lhs_k[kb][:, bi * P:(bi + 1) * P],
                        rhs=W2r[:, kb * HID4:(kb + 1) * HID4],
                        start=(kb == 0), stop=(kb == 3))
                y2c = wp.tile([P, HID4], BF16, tag="y2c", name="y2c")
                nc.vector.tensor_copy(y2c[:], ps[:])
                r0 = n0 + bi * P
                nc.sync.dma_start(y2nm[r0:r0 + P, :], y2c[:])
            if s in AG_TRIG:
                q = AG_TRIG[s]
                nc.gpsimd.collective_compute(
                    "AllGather", OP.bypass, replica_groups=rg,
                    ins=[y2nm[q * QSH:(q + 1) * QSH, :]],
                    outs=[table2[q * QVP:(q + 1) * QVP, :]])

        QSH = SHP // 4
        QVP = VP // 4
        AG_TRIG = {2: 0, 4: 1, 7: 2}

        _conv_scatter(tc, pools, cdat, None, IN_DIM, True, None, p1_post_sb,
                      stream_src=ins["xe"][:])

        nc.gpsimd.collective_compute(
            "AllGather", OP.bypass, replica_groups=rg,
            ins=[y2nm[3 * QSH:, :]], outs=[table2[3 * QVP:, :]])

        # =========== phase 2: conv2 + y3' ===========
        def p2_post_sb(s, agg_ps):
            n0 = s * CHUNK
            x2c = [chp.tile([P, CHUNK], F32, tag="x2c", name=f"x2c{db}") for db in range(2)]
            for db in range(2):
                nc.scalar.activation(x2c[db][:], agg_ps[db][:], AF.Relu)
            for bi in range(SBB):
                ps = dp.tile([P, OUT_DIM], F32, space="PSUM", tag="dps",
                             name="y3ps")
                for kb in range(2):
                    nc.tensor.matmul(
                        ps[:], lhsT=x2c[kb][:, bi * P:(bi + 1) * P],
                        rhs=W3r[:, kb * OUT_DIM:(kb + 1) * OUT_DIM],
                        start=(kb == 0), stop=(kb == 1))
                y3c = wp.tile([P, OUT_DIM], BF16, tag="y3c", name="y3c")
                nc.vector.tensor_copy(y3c[:], ps[:])
                r0 = n0 + bi * P
                nc.sync.dma_start(y3nm[r0:r0 + P, :], y3c[:])
            if s in AG_TRIG:
                q = AG_TRIG[s]
                nc.gpsimd.collective_compute(
                    "AllGather", OP.bypass, replica_groups=rg,
                    ins=[y3nm[q * QSH:(q + 1) * QSH, :]],
                    outs=[table3[q * QVP:(q + 1) * QVP, :]])

        _conv_scatter(tc, pools, cdat, table2[:], HID4, True, None, p2_post_sb)

        nc.gpsimd.collective_compute(
            "AllGather", OP.bypass, replica_groups=rg,
            ins=[y3nm[3 * QSH:, :]], outs=[table3[3 * QVP:, :]])

        # =========== phase 3: conv3 (node-major) + readout ===========
        ro_ps = rop.tile([N_GRAPHS, OUT_DIM], F32, space="PSUM")

        def p3_post_block(s, bi, agg_nm):
            b = s * SBB + bi
            x3 = wp.tile([P, OUT_DIM], F32, tag="x3")
            nc.scalar.activation(x3[:], agg_nm[:], AF.Relu)
            goh = wp.tile([P, N_GRAPHS], F32, tag="goh")
            nc.vector.tensor_scalar(
                out=goh[:], in0=iota_sb[:, :N_GRAPHS],
                scalar1=gid_sb[:, b:b + 1], scalar2=None, op0=OP.is_equal)
            nc.tensor.matmul(ro_ps[:], lhsT=goh[:], rhs=x3[:],
                             start=(b == 0), stop=(b == NBLK - 1))

        _conv_scatter(tc, pools, cdat, table3[:], OUT_DIM, False,
                      p3_post_block, None)

        # readout allreduce + normalize
        ro_sb = wp.tile([N_GRAPHS, OUT_DIM], F32, tag="ro")
        nc.vector.tensor_copy(ro_sb[:], ro_ps[:])
        nc.gpsimd.dma_start(ro_in[:], ro_sb[:])
        nc.gpsimd.collective_compute(
            "AllReduce", OP.add, replica_groups=rg,
            ins=[ro_in[:]], outs=[ro_out[:]])
        r = wp.tile([N_GRAPHS, OUT_DIM], F32, tag="r")
        nc.sync.dma_start(r[:], ro_out[:])
        sq = wp.tile([N_GRAPHS, OUT_DIM], F32, tag="rsq")
        nc.vector.tensor_tensor(out=sq[:], in0=r[:], in1=r[:], op=OP.mult)
        ssq = wp.tile([N_GRAPHS, 1], F32, tag="rssq")
        nc.vector.tensor_reduce(out=ssq[:], in_=sq[:],
                                axis=mybir.AxisListType.X, op=OP.add)
        nrm = wp.tile([N_GRAPHS, 1], F32, tag="rnrm")
        nc.scalar.activation(nrm[:], ssq[:], AF.Sqrt)
        nc.vector.tensor_scalar(out=nrm[:], in0=nrm[:], scalar1=1e-12,
                                scalar2=None, op0=OP.max)
        rn = wp.tile([N_GRAPHS, 1], F32, tag="rrn")
        nc.vector.reciprocal(rn[:], nrm[:])
        o = wp.tile([N_GRAPHS, OUT_DIM], F32, tag="ofin")
        nc.vector.tensor_scalar(out=o[:], in0=r[:], scalar1=rn[:, :1],
                                scalar2=None, op0=OP.mult)
        nc.sync.dma_start(out_ap, o[:])


# ======================= top-level entry =======================

_CACHE = {}

IN_SPECS = {
    "xe": ((P, NT * IN_DIM), BF),
    "iota": ((P, P), np.float32),
    "ones_col": ((P, 1), np.float32),
    "ones_row": ((1, P), np.float32),
    "W1": ((IN_DIM, HID4), np.float32),
    "fc1_W": ((IN_DIM, HID4), np.float32),
    "W2r": ((P, 4 * HID4), np.float32),
    "W3r": ((P, 2 * OUT_DIM), np.float32),
    "gammaT": ((P, 2), np.float32),
    "betaT": ((P, 2), np.float32),
    "gidx": ((P, NT), np.int32),
    "dstl": ((P, NT), np.float32),
    "q": ((P, NT), np.float32),
    "wdat": ((P, NT * 4), np.float32),
    "gid": ((P, NBLK), np.float32),
    "xT": ((IN_DIM, SHP), np.float32),
}
OUT_SPECS = {"out": ((N_GRAPHS, OUT_DIM), np.float32)}


def _build_nc():
    if "nc" in _CACHE:
        return _CACHE["nc"]
    nc = bacc.Bacc("TRN2", target_bir_lowering=False, debug=False,
                   num_devices=NCORES)
    ins = {}
    _DT = {np.dtype(np.float32): F32, np.dtype(np.int32): I32,
           np.dtype(BF): BF16}
    for name, (shape, dt) in IN_SPECS.items():
        ins[name] = nc.dram_tensor(name, list(shape), _DT[np.dtype(dt)],
                                   kind="ExternalInput").ap()
    outs = {}
    for name, (shape, dt) in OUT_SPECS.items():
        outs[name] = nc.dram_tensor(name, list(shape), _DT[np.dtype(dt)],
                                    kind="ExternalOutput").ap()
    with tile.TileContext(nc) as tc:
        build_kernel(tc, ins, outs)
    nc.compile()
    _CACHE["nc"] = nc
    return nc


LAST_EXEC_NS = None


def make_in_maps(x, w, W1, fc1_W, ln_gamma, ln_beta, W2, W3, src, dst,
                 graph_ids):
    shared, per_core = _preprocess(x, w, src, dst, graph_ids)
    W1 = np.ascontiguousarray(W1, np.float32)
    fc1_W = np.ascontiguousarray(fc1_W, np.float32)
    W2 = np.asarray(W2, np.float32)
    W3 = np.asarray(W3, np.float32)
    W2r = W2.reshape(4, P, HID4).transpose(1, 0, 2).reshape(P, 4 * HID4)
    W3r = W3.reshape(2, P, OUT_DIM).transpose(1, 0, 2).reshape(P, 2 * OUT_DIM)
    W2r = np.ascontiguousarray(W2r)
    W3r = np.ascontiguousarray(W3r)
    gammaT = np.ascontiguousarray(
        np.asarray(ln_gamma, np.float32).reshape(2, P).T)
    betaT = np.ascontiguousarray(
        np.asarray(ln_beta, np.float32).reshape(2, P).T)
    in_maps = []
    for c in range(NCORES):
        pc = per_core[c]
        in_maps.append({
            "xe": pc["xe"], "iota": shared["iota"],
            "ones_col": shared["ones_col"], "ones_row": shared["ones_row"],
            "W1": W1, "fc1_W": fc1_W, "W2r": W2r, "W3r": W3r,
            "gammaT": gammaT, "betaT": betaT,
            "gidx": pc["gidx"], "dstl": pc["dstl"], "q": pc["q"],
            "wdat": pc["wdat"], "gid": pc["gid"], "xT": pc["xT"],
        })
    return in_maps


def kernel(x, w, W1, fc1_W, ln_gamma, ln_beta, W2, W3, src, dst, graph_ids):
    global LAST_EXEC_NS
    x = np.asarray(x, np.float32)
    w = np.asarray(w, np.float32)
    in_maps = make_in_maps(x, w, W1, fc1_W, ln_gamma, ln_beta, W2, W3,
                           src, dst, graph_ids)
    nc = _build_nc()
    trace = os.environ.get("GCN_TRACE", "0") == "1"
    res = bass_utils.run_bass_kernel_spmd(
        nc, in_maps, core_ids=list(range(NCORES)), trace=trace)
    LAST_EXEC_NS = res.exec_time_ns
    return np.asarray(res.results[0]["out"], np.float32)



# revision 2
# speedup vs baseline: 2.0022x; 2.0022x over previous
"""GCN message-passing kernel for 8 Trainium2 NeuronCores (Bass/Tile).

Strategy (SPMD, one program for all 8 cores):
  - Nodes sharded contiguously: core c owns nodes [5000c, 5000(c+1)), padded
    to 5120 (40 blocks of 128).  Within the shard, nodes are permuted by a
    greedy load-balancer so every (core, block) has bounded in-degree.
  - Edges assigned to the core owning their dst, grouped into 128-edge tiles
    per dst-block (8 tiles per block).
  - Gather: per-tile indirect DMA pulls 128 message rows from a replicated
    node-major table in HBM (bf16).  Scatter: one-hot matmul on the
    TensorEngine accumulating into PSUM (edge-tile stationary = transposed
    output for conv1/2; one-hot stationary = node-major output for conv3).
  - All per-edge scalars (ew * in_inv[dst] * out_inv[src]) fold into the
    one-hot coefficient.  GraphConv weight is applied before propagation
    when it shrinks the message (conv2: 512->256, conv3: 256->128).
  - Dense transforms run feature-major with the weights stationary.
    LayerNorm stats use ones-matmul partition reduction + K=1 broadcast.
  - Tables AllGather'd between convs; readout AllReduce'd; final L2
    normalize computed identically on every core.
"""
import os
import numpy as np
import ml_dtypes

import concourse.bacc as bacc
import concourse.bass as bass
import concourse.tile as tile
import concourse.mybir as mybir
import concourse.bass_utils as bass_utils
from concourse.bass import IndirectOffsetOnAxis

# ---------------- problem constants (hardcoded per spec) ----------------
N_NODES = 40000
N_EDGES = 320000
N_GRAPHS = 64
IN_DIM = 128
HID4 = 256
OUT_DIM = 128
LN_EPS = 1e-5

NCORES = 8
SH = N_NODES // NCORES          # 5000 nodes per core
NBLK = 40                       # 128-node blocks per core
P = 128
SHP = NBLK * P                  # 5120 padded nodes per core
VP = NCORES * SHP               # 40960 padded global rows
TB = 8                          # tiles per block
SBB = 4                         # blocks per superblock (gather chunk)
NSB = NBLK // SBB               # 10 superblocks
CHUNK = SBB * P                 # 512 nodes per dense chunk
TILES_SB = SBB * TB             # 32 tiles per superblock
NT = NBLK * TB                  # 320 tiles per core

F32 = mybir.dt.float32
BF16 = mybir.dt.bfloat16
I32 = mybir.dt.int32
BF = ml_dtypes.bfloat16

AF = mybir.ActivationFunctionType
OP = mybir.AluOpType


# ======================= host-side preprocessing =======================

def _preprocess(x, w, src, dst, graph_ids):
    x = np.asarray(x, np.float32)
    w = np.asarray(w, np.float32)
    src = np.asarray(src, np.int64)
    dst = np.asarray(dst, np.int64)
    graph_ids = np.asarray(graph_ids, np.int64)

    deg_out = np.bincount(src, minlength=N_NODES).astype(np.float64)
    deg_in = np.bincount(dst, minlength=N_NODES).astype(np.float64)
    out_inv = (1.0 / np.sqrt(np.maximum(deg_out, 1.0))).astype(np.float32)
    in_inv = (1.0 / np.sqrt(np.maximum(deg_in, 1.0))).astype(np.float32)

    # ---- per-core node -> (block, local) assignment, balancing in-degree ----
    slot_of = np.full(N_NODES, -1, np.int64)     # slot in [0, SHP) within shard
    for c in range(NCORES):
        lo, hi = c * SH, (c + 1) * SH
        em = (dst >= lo) & (dst < hi)
        tot = np.bincount(dst[em] - lo, minlength=SH)
        order = np.argsort(-tot, kind="stable")
        loads = np.zeros(NBLK, np.int64)
        counts = np.zeros(NBLK, np.int64)
        blk = np.empty(SH, np.int64)
        loc = np.empty(SH, np.int64)
        for v in order:
            masked = np.where(counts < P, loads, 1 << 60)
            b = int(np.argmin(masked))
            blk[v] = b
            loc[v] = counts[b]
            counts[b] += 1
            loads[b] += tot[v]
        assert loads.max() <= TB * P, f"core {c}: max block load {loads.max()}"
        slot_of[lo:hi] = blk * P + loc

    core_of = np.arange(N_NODES) // SH
    allslot = slot_of[np.arange(N_NODES)]
    QSH = SHP // 4
    quart = allslot // QSH
    # quarter-major layout: AllGather of each shard-quarter writes one slice
    rowp = quart * (VP // 4) + core_of * QSH + (allslot % QSH)

    # ---- replicated inputs ----
    x_bf = np.zeros((VP, IN_DIM), BF)
    x_bf[rowp] = x.astype(BF)
    iota128 = np.tile(np.arange(P, dtype=np.float32), (P, 1))
    ones_col = np.ones((P, 1), np.float32)
    ones_row = np.ones((1, P), np.float32)

    per_core = []
    for c in range(NCORES):
        lo, hi = c * SH, (c + 1) * SH
        em_idx = np.nonzero((dst >= lo) & (dst < hi))[0]
        e_dst = dst[em_idx]
        e_slot = slot_of[e_dst]
        e_blk = e_slot // P
        e_dl = (e_slot % P).astype(np.float32)
        e_row = rowp[src[em_idx]]
        e_q = in_inv[e_dst] * out_inv[src[em_idx]]

        # order edges by block, then place into slot grid [p, t]
        order = np.argsort(e_blk, kind="stable")
        gidx = np.zeros((P, NT), np.int32)
        dstl = np.zeros((P, NT), np.float32)
        qv = np.zeros((P, NT), np.float32)
        wdat = np.zeros((P, NT, 4), np.float32)
        bstart = np.searchsorted(e_blk[order], np.arange(NBLK + 1))
        for b in range(NBLK):
            sel = order[bstart[b]:bstart[b + 1]]
            k = np.arange(len(sel))
            t = b * TB + k // P
            p = k % P
            gidx[p, t] = e_row[sel]
            dstl[p, t] = e_dl[sel]
            qv[p, t] = e_q[sel]
            wdat[p, t] = w[em_idx[sel]]

        nodes = np.arange(lo, hi)
        slots = slot_of[nodes]
        xT = np.zeros((IN_DIM, SHP), np.float32)
        xT[:, slots] = x[nodes].T
        gid = np.zeros((P, NBLK), np.float32)
        gid[slots % P, slots // P] = graph_ids[nodes]

        xe = x_bf[gidx.reshape(-1)].reshape(P, NT * IN_DIM)
        per_core.append(dict(
            gidx=gidx, dstl=dstl, q=qv,
            wdat=wdat.reshape(P, NT * 4),
            xT=xT, gid=gid, xe=xe,
        ))
    shared = dict(x_bf=x_bf, iota=iota128, ones_col=ones_col,
                  ones_row=ones_row)
    return shared, per_core


# ======================= device kernel =======================

def _conv_scatter(tc, pools, cdat, table, dnum, transposed,
                  post_block, post_sb, dtt=BF16, stream_src=None):
    """Shared conv loop: gathers (indirect, or a contiguous host-expanded
    stream) + one-hot scatter matmuls."""
    nc = tc.nc
    gp, ohp, aggp = pools["gather"], pools["onehot"], pools["agg"]
    gidx, dstl_sb, c_sb, iota_sb = (cdat["gidx"], cdat["dstl"],
                                    cdat["c"], cdat["iota"])
    ndb = dnum // P  # feature blocks per message
    for s in range(NSB):
        g = gp.tile([P, TILES_SB, dnum], dtt, tag="gbuf")
        if stream_src is not None:
            w0 = s * TILES_SB * dnum
            nc.sync.dma_start(g[:].rearrange("p t d -> p (t d)"),
                              stream_src[:, w0:w0 + TILES_SB * dnum])
        else:
            # One SWDGE call per superblock: 32 tiles x 128 rows = 4096
            # descriptors.  SWDGE cost is ~994ns fixed + 0.34ns/desc, so
            # batching beats 32 per-tile calls by ~30us per superblock.
            t0 = s * TILES_SB
            nc.gpsimd.indirect_dma_start(
                out=g[:].rearrange("p t d -> p (t d)"), out_offset=None,
                in_=table,
                in_offset=IndirectOffsetOnAxis(
                    ap=gidx[:, t0:t0 + TILES_SB], axis=0))
        if transposed:
            agg = [aggp.tile([P, SBB * P], F32, space="PSUM", tag="agg",
                             name=f"agg{db}") for db in range(ndb)]
        for bi in range(SBB):
            b = s * SBB + bi
            if not transposed:
                agg = aggp.tile([P, P], F32, space="PSUM", tag="agg",
                                name="aggnm")
            for j in range(TB):
                sl = bi * TB + j
                t_col = s * TILES_SB + sl
                oh = ohp.tile([P, P], dtt, tag="oh")
                nc.vector.tensor_scalar(
                    out=oh[:], in0=iota_sb[:, :P],
                    scalar1=dstl_sb[:, t_col:t_col + 1],
                    scalar2=c_sb[:, t_col:t_col + 1],
                    op0=OP.is_equal, op1=OP.mult)
                msg = g[:, sl, :]
                if transposed:
                    for db in range(ndb):
                        nc.tensor.matmul(
                            agg[db][:, bi * P:(bi + 1) * P],
                            lhsT=msg[:, db * P:(db + 1) * P], rhs=oh[:],
                            start=(j == 0), stop=(j == TB - 1))
                else:
                    nc.tensor.matmul(
                        agg[:], lhsT=oh[:], rhs=msg,
                        start=(j == 0), stop=(j == TB - 1))
            if post_block is not None:
                post_block(s, bi, agg)
        if post_sb is not None:
            post_sb(s, agg)


def build_kernel(tc, ins, outs):
    nc = tc.nc
    out_ap = outs["out"][:]

    # internal DRAM tensors
    y2nm = nc.dram_tensor("y2nm", [SHP, HID4], BF16, kind="Internal").ap()
    y3nm = nc.dram_tensor("y3nm", [SHP, OUT_DIM], BF16, kind="Internal").ap()
    table2 = nc.dram_tensor("table2", [VP, HID4], BF16, kind="Internal",
                            addr_space="Shared").ap()
    table3 = nc.dram_tensor("table3", [VP, OUT_DIM], BF16, kind="Internal",
                            addr_space="Shared").ap()
    ro_in = nc.dram_tensor("ro_in", [N_GRAPHS, OUT_DIM], F32,
                           kind="Internal").ap()
    ro_out = nc.dram_tensor("ro_out", [N_GRAPHS, OUT_DIM], F32,
                            kind="Internal", addr_space="Shared").ap()
    rg = [list(range(NCORES))]

    with tc.tile_pool(name="const", bufs=1) as cp, \
         tc.tile_pool(name="gather", bufs=3) as gp, \
         tc.tile_pool(name="onehot", bufs=8) as ohp, \
         tc.tile_pool(name="work", bufs=2) as wp, \
         tc.tile_pool(name="chunk", bufs=2) as chp, \
         tc.tile_pool(name="agg", bufs=2, space="PSUM") as aggp, \
         tc.tile_pool(name="dense", bufs=4, space="PSUM") as dp, \
         tc.tile_pool(name="stats", bufs=1, space="PSUM") as sp, \
         tc.tile_pool(name="ro", bufs=1, space="PSUM") as rop:

        pools = dict(gather=gp, onehot=ohp, agg=aggp)

        # ---- load constants ----
        def cload(name, shape, dt):
            t = cp.tile(shape, dt, name=name, tag=name)
            nc.sync.dma_start(t[:], ins[name][:])
            return t

        iota_sb = cload("iota", [P, P], F32)
        onesc = cload("ones_col", [P, 1], F32)
        onesr = cload("ones_row", [1, P], F32)
        W1 = cload("W1", [IN_DIM, HID4], F32)
        fc1W = cload("fc1_W", [IN_DIM, HID4], F32)
        W2r = cload("W2r", [P, 4 * HID4], F32)
        W3r = cload("W3r", [P, 2 * OUT_DIM], F32)
        gammaT = cload("gammaT", [P, 2], F32)
        betaT = cload("betaT", [P, 2], F32)
        gidx_sb = cload("gidx", [P, NT], I32)
        dstl_sb = cload("dstl", [P, NT], F32)
        q_sb = cload("q", [P, NT], F32)
        wdat_sb = cload("wdat", [P, NT * 4], F32)
        gid_sb = cload("gid", [P, NBLK], F32)
        xT_dram = ins["xT"]

        # ---- c = max(w, axis=1) * q ----
        eps_t = cp.tile([1, 1], F32)
        nc.vector.memset(eps_t[:], LN_EPS)

        ew_sb = cp.tile([P, NT], F32)
        nc.vector.tensor_reduce(
            out=ew_sb[:], in_=wdat_sb[:].rearrange("p (t f) -> p t f", f=4),
            axis=mybir.AxisListType.X, op=OP.max)
        c_sb = cp.tile([P, NT], F32)
        nc.vector.tensor_tensor(out=c_sb[:], in0=ew_sb[:], in1=q_sb[:],
                                op=OP.mult)

        cdat = dict(gidx=gidx_sb[:], dstl=dstl_sb[:], c=c_sb[:],
                    iota=iota_sb[:])

        wbar = cp.tile([P, 1], F32, name="wbar", tag="wbar")
        nc.vector.tensor_reduce(out=wbar[:], in_=fc1W[:],
                                axis=mybir.AxisListType.X, op=OP.add)

        # =========== phase 1: conv1 + fc1 + y2' (fused per superblock) =====
        def p1_post_sb(s, agg_ps):
            n0 = s * CHUNK
            # conv1 agg -> SBUF
            a1 = wp.tile([P, CHUNK], F32, tag="a1")
            nc.vector.tensor_copy(a1[:], agg_ps[0][:])
            # x1T = relu(W1^T @ a1)  (2 feature blocks)
            x1c = [chp.tile([P, CHUNK], F32, tag="x1c", name=f"x1c{ob}") for ob in range(2)]
            for ob in range(2):
                ps = dp.tile([P, CHUNK], F32, space="PSUM", tag="dps")
                nc.tensor.matmul(ps[:], lhsT=W1[:, ob * P:(ob + 1) * P],
                                 rhs=a1[:], start=True, stop=True)
                nc.scalar.activation(x1c[ob][:], ps[:], AF.Relu)
            # fc1 chunk
            xTc = wp.tile([P, CHUNK], F32, tag="xTc")
            nc.sync.dma_start(xTc[:], xT_dram[:, n0:n0 + CHUNK])
            fpre = [dp.tile([P, CHUNK], F32, space="PSUM", tag="dps", name=f"fpre{ob}")
                    for ob in range(2)]
            fps = [wp.tile([P, CHUNK], F32, tag="fp", name=f"fp{ob}") for ob in range(2)]
            sqs = [wp.tile([P, CHUNK], F32, tag="sq", name=f"sq{ob}") for ob in range(2)]
            for ob in range(2):
                nc.tensor.matmul(fpre[ob][:], lhsT=fc1W[:, ob * P:(ob + 1) * P],
                                 rhs=xTc[:], start=True, stop=True)
                nc.scalar.copy(fps[ob][:], fpre[ob][:])
                nc.vector.tensor_tensor(out=sqs[ob][:], in0=fps[ob][:],
                                        in1=fps[ob][:], op=OP.mult)
            srow = wp.tile([1, 2 * CHUNK], F32, tag="srow")
            stats = sp.tile([1, CHUNK], F32, space="PSUM", tag="stats",
                            name="stats_s")
            nc.tensor.matmul(stats[:], lhsT=wbar[:], rhs=xTc[:],
                             start=True, stop=True)
            nc.vector.tensor_copy(srow[:, :CHUNK], stats[:])
            stats2 = rop.tile([1, CHUNK], F32, space="PSUM", tag="ro_ps",
                              name="stats_ss")
            for ob in range(2):
                nc.tensor.matmul(stats2[:], lhsT=onesc[:], rhs=sqs[ob][:],
                                 start=(ob == 0), stop=(ob == 1))
            nc.vector.tensor_copy(srow[:, CHUNK:], stats2[:])
            # lane-0 stats math
            mu1 = wp.tile([1, CHUNK], F32, tag="mu1")
            var1 = wp.tile([1, CHUNK], F32, tag="var1")
            nc.vector.tensor_scalar(out=mu1[:], in0=srow[:, :CHUNK],
                                    scalar1=1.0 / HID4, scalar2=None,
                                    op0=OP.mult)
            nc.vector.tensor_scalar(out=var1[:], in0=srow[:, CHUNK:],
                                    scalar1=1.0 / HID4, scalar2=None,
                                    op0=OP.mult)
            musq = wp.tile([1, CHUNK], F32, tag="musq")
            nc.vector.tensor_tensor(out=musq[:], in0=mu1[:], in1=mu1[:],
                                    op=OP.mult)
            nc.vector.tensor_tensor(out=var1[:], in0=var1[:], in1=musq[:],
                                    op=OP.subtract)
            lnv = wp.tile([1, CHUNK], F32, tag="lnv")
            nc.scalar.activation(lnv[:], var1[:], AF.Ln, bias=eps_t[:1, :1])
            rstd1 = wp.tile([1, CHUNK], F32, tag="rstd1")
            nc.scalar.activation(rstd1[:], lnv[:], AF.Exp, scale=-0.5)
            # broadcast mu and rstd to 128 partitions
            mub = wp.tile([P, CHUNK], F32, tag="mub")
            rstdb = wp.tile([P, CHUNK], F32, tag="rstdb")
            bcm = dp.tile([P, CHUNK], F32, space="PSUM", tag="dps",
                          name="bcm")
            nc.tensor.matmul(bcm[:], lhsT=onesr[:], rhs=mu1[:],
                             start=True, stop=True)
            nc.scalar.copy(mub[:], bcm[:])
            bcr = dp.tile([P, CHUNK], F32, space="PSUM", tag="dps",
                          name="bcr")
            nc.tensor.matmul(bcr[:], lhsT=onesr[:], rhs=rstd1[:],
                             start=True, stop=True)
            nc.vector.tensor_copy(rstdb[:], bcr[:])
            f1c = [chp.tile([P, CHUNK], F32, tag="f1c", name=f"f1c{ob}") for ob in range(2)]
            for ob in range(2):
                d = wp.tile([P, CHUNK], F32, tag="lnd")
                nc.vector.tensor_tensor(out=d[:], in0=fps[ob][:], in1=mub[:],
                                        op=OP.subtract)
                nc.vector.tensor_tensor(out=d[:], in0=d[:], in1=rstdb[:],
                                        op=OP.mult)
                nc.scalar.activation(f1c[ob][:], d[:], AF.Relu,
                                     bias=betaT[:, ob:ob + 1],
                                     scale=gammaT[:, ob:ob + 1])
            # y2' node-major: per node-block, x1f1^T blocks stationary
            lhs_k = [x1c[0], x1c[1], f1c[0], f1c[1]]
            for bi in range(SBB):
                ps = dp.tile([P, HID4], F32, space="PSUM", tag="dps",
                             name="y2ps")
                for kb in range(4):
                    nc.tensor.matmul(
                        ps[:], lhsT=lhs_k[kb][:, bi * P:(bi + 1) * P],
                        rhs=W2r[:, kb * HID4:(kb + 1) * HID4],
                        start=(kb == 0), stop=(kb == 3))
                y2c = wp.tile([P, HID4], BF16, tag="y2c", name="y2c")
                nc.vector.tensor_copy(y2c[:], ps[:])
                r0 = n0 + bi * P
                nc.sync.dma_start(y2nm[r0:r0 + P, :], y2c[:])
            if s in AG_TRIG:
                q = AG_TRIG[s]
                nc.gpsimd.collective_compute(
                    "AllGather", OP.bypass, replica_groups=rg,
                    ins=[y2nm[q * QSH:(q + 1) * QSH, :]],
                    outs=[table2[q * QVP:(q + 1) * QVP, :]])

        QSH = SHP // 4
        QVP = VP // 4
        AG_TRIG = {2: 0, 4: 1, 7: 2}

        _conv_scatter(tc, pools, cdat, None, IN_DIM, True, None, p1_post_sb,
                      stream_src=ins["xe"][:])

        nc.gpsimd.collective_compute(
            "AllGather", OP.bypass, replica_groups=rg,
            ins=[y2nm[3 * QSH:, :]], outs=[table2[3 * QVP:, :]])

        # =========== phase 2: conv2 + y3' ===========
        def p2_post_sb(s, agg_ps):
            n0 = s * CHUNK
            x2c = [chp.tile([P, CHUNK], F32, tag="x2c", name=f"x2c{db}") for db in range(2)]
            for db in range(2):
                nc.scalar.activation(x2c[db][:], agg_ps[db][:], AF.Relu)
            for bi in range(SBB):
                ps = dp.tile([P, OUT_DIM], F32, space="PSUM", tag="dps",
                             name="y3ps")
                for kb in range(2):
                    nc.tensor.matmul(
                        ps[:], lhsT=x2c[kb][:, bi * P:(bi + 1) * P],
                        rhs=W3r[:, kb * OUT_DIM:(kb + 1) * OUT_DIM],
                        start=(kb == 0), stop=(kb == 1))
                y3c = wp.tile([P, OUT_DIM], BF16, tag="y3c", name="y3c")
                nc.vector.tensor_copy(y3c[:], ps[:])
                r0 = n0 + bi * P
                nc.sync.dma_start(y3nm[r0:r0 + P, :], y3c[:])
            if s in AG_TRIG:
                q = AG_TRIG[s]
                nc.gpsimd.collective_compute(
                    "AllGather", OP.bypass, replica_groups=rg,
                    ins=[y3nm[q * QSH:(q + 1) * QSH, :]],
                    outs=[table3[q * QVP:(q + 1) * QVP, :]])

        _conv_scatter(tc, pools, cdat, table2[:], HID4, True, None, p2_post_sb)

        nc.gpsimd.collective_compute(
            "AllGather", OP.bypass, replica_groups=rg,
            ins=[y3nm[3 * QSH:, :]], outs=[table3[3 * QVP:, :]])

        # =========== phase 3: conv3 (node-major) + readout ===========
        ro_ps = rop.tile([N_GRAPHS, OUT_DIM], F32, space="PSUM")

        def p3_post_block(s, bi, agg_nm):
            b = s * SBB + bi
            x3 = wp.tile([P, OUT_DIM], F32, tag="x3")
            nc.scalar.activation(x3[:], agg_nm[:], AF.Relu)
            goh = wp.tile([P, N_GRAPHS], F32, tag="goh")
            nc.vector.tensor_scalar(
                out=goh[:], in0=iota_sb[:, :N_GRAPHS],
                scalar1=gid_sb[:, b:b + 1], scalar2=None, op0=OP.is_equal)
            nc.tensor.matmul(ro_ps[:], lhsT=goh[:], rhs=x3[:],
                             start=(b == 0), stop=(b == NBLK - 1))

        _conv_scatter(tc, pools, cdat, table3[:], OUT_DIM, False,
                      p3_post_block, None)

        # readout allreduce + normalize
        ro_sb = wp.tile([N_GRAPHS, OUT_DIM], F32, tag="ro")
        nc.vector.tensor_copy(ro_sb[:], ro_ps[:])
        nc.gpsimd.dma_start(ro_in[:], ro_sb[:])
        nc.gpsimd.collective_compute(
            "AllReduce", OP.add, replica_groups=rg,
            ins=[ro_in[:]], outs=[ro_out[:]])
        r = wp.tile([N_GRAPHS, OUT_DIM], F32, tag="r")
        nc.sync.dma_start(r[:], ro_out[:])
        sq = wp.tile([N_GRAPHS, OUT_DIM], F32, tag="rsq")
        nc.vector.tensor_tensor(out=sq[:], in0=r[:], in1=r[:], op=OP.mult)
        ssq = wp.tile([N_GRAPHS, 1], F32, tag="rssq")
        nc.vector.tensor_reduce(out=ssq[:], in_=sq[:],
                                axis=mybir.AxisListType.X, op=OP.add)
        nrm = wp.tile([N_GRAPHS, 1], F32, tag="rnrm")
        nc.scalar.activation(nrm[:], ssq[:], AF.Sqrt)
        nc.vector.tensor_scalar(out=nrm[:], in0=nrm[:], scalar1=1e-12,
                                scalar2=None, op0=OP.max)
        rn = wp.tile([N_GRAPHS, 1], F32, tag="rrn")
        nc.vector.reciprocal(rn[:], nrm[:])
        o = wp.tile([N_GRAPHS, OUT_DIM], F32, tag="ofin")
        nc.vector.tensor_scalar(out=o[:], in0=r[:], scalar1=rn[:, :1],
                                scalar2=None, op0=OP.mult)
        nc.sync.dma_start(out_ap, o[:])


# ======================= top-level entry =======================

_CACHE = {}

IN_SPECS = {
    "xe": ((P, NT * IN_DIM), BF),
    "iota": ((P, P), np.float32),
    "ones_col": ((P, 1), np.float32),
    "ones_row": ((1, P), np.float32),
    "W1": ((IN_DIM, HID4), np.float32),
    "fc1_W": ((IN_DIM, HID4), np.float32),
    "W2r": ((P, 4 * HID4), np.float32),
    "W3r": ((P, 2 * OUT_DIM), np.float32),
    "gammaT": ((P, 2), np.float32),
    "betaT": ((P, 2), np.float32),
    "gidx": ((P, NT), np.int32),
    "dstl": ((P, NT), np.float32),
    "q": ((P, NT), np.float32),
    "wdat": ((P, NT * 4), np.float32),
    "gid": ((P, NBLK), np.float32),
    "xT": ((IN_DIM, SHP), np.float32),
}
OUT_SPECS = {"out": ((N_GRAPHS, OUT_DIM), np.float32)}


def _build_nc():
    if "nc" in _CACHE:
        return _CACHE["nc"]
    nc = bacc.Bacc("TRN2", target_bir_lowering=False, debug=False,
                   num_devices=NCORES)
    ins = {}
    _DT = {np.dtype(np.float32): F32, np.dtype(np.int32): I32,
           np.dtype(BF): BF16}
    for name, (shape, dt) in IN_SPECS.items():
        ins[name] = nc.dram_tensor(name, list(shape), _DT[np.dtype(dt)],
                                   kind="ExternalInput").ap()
    outs = {}
    for name, (shape, dt) in OUT_SPECS.items():
        outs[name] = nc.dram_tensor(name, list(shape), _DT[np.dtype(dt)],
                                    kind="ExternalOutput").ap()
    with tile.TileContext(nc) as tc:
        build_kernel(tc, ins, outs)
    nc.compile()
    _CACHE["nc"] = nc
    return nc


LAST_EXEC_NS = None


def make_in_maps(x, w, W1, fc1_W, ln_gamma, ln_beta, W2, W3, src, dst,
                 graph_ids):
    shared, per_core = _preprocess(x, w, src, dst, graph_ids)
    W1 = np.ascontiguousarray(W1, np.float32)
    fc1_W = np.ascontiguousarray(fc1_W, np.float32)
    W2 = np.asarray(W2, np.float32)
    W3 = np.asarray(W3, np.float32)
    W2r = W2.reshape(4, P, HID4).transpose(1, 0, 2).reshape(P, 4 * HID4)
    W3r = W3.reshape(2, P, OUT_DIM).transpose(1, 0, 2).reshape(P, 2 * OUT_DIM)
    W2r = np.ascontiguousarray(W2r)
    W3r = np.ascontiguousarray(W3r)
    gammaT = np.ascontiguousarray(
        np.asarray(ln_gamma, np.float32).reshape(2, P).T)
    betaT = np.ascontiguousarray(
        np.asarray(ln_beta, np.float32).reshape(2, P).T)
    in_maps = []
    for c in range(NCORES):
        pc = per_core[c]
        in_maps.append({
            "xe": pc["xe"], "iota": shared["iota"],
            "ones_col": shared["ones_col"], "ones_row": shared["ones_row"],
            "W1": W1, "fc1_W": fc1_W, "W2r": W2r, "W3r": W3r,
            "gammaT": gammaT, "betaT": betaT,
            "gidx": pc["gidx"], "dstl": pc["dstl"], "q": pc["q"],
            "wdat": pc["wdat"], "gid": pc["gid"], "xT": pc["xT"],
        })
    return in_maps


def kernel(x, w, W1, fc1_W, ln_gamma, ln_beta, W2, W3, src, dst, graph_ids):
    global LAST_EXEC_NS
    x = np.asarray(x, np.float32)
    w = np.asarray(w, np.float32)
    in_maps = make_in_maps(x, w, W1, fc1_W, ln_gamma, ln_beta, W2, W3,
                           src, dst, graph_ids)
    nc = _build_nc()
    trace = os.environ.get("GCN_TRACE", "0") == "1"
    res = bass_utils.run_bass_kernel_spmd(
        nc, in_maps, core_ids=list(range(NCORES)), trace=trace)
    LAST_EXEC_NS = res.exec_time_ns
    return np.asarray(res.results[0]["out"], np.float32)

